# revision 7
# baseline (speedup 1.0000x reference)
"""Trainium2 Bass kernel for the ConvPolicy30 policy network.

Contract: kernel(**inputs) takes the full unsharded inputs (x + all conv
weights, numpy arrays) and returns the full (1, 88) output.

Strategy (per sharding hint): the network is far too small to shard, so the
whole forward pass runs on every core (replicated inputs, data-parallel-ready);
core 0's output is returned.

Device-side design: one serial PE(matmul) <-> ACT(tanh) chain.
 - conv1 is a single matmul over an im2col patch built by one strided DMA
   from a host-packed [0,0,x7..x94, 0,0,x101..x188] vector (layout only).
 - the adaptive-avg-pool 13->5 is a matmul against a constant matrix.
 - every conv bias is folded into its matmul via an all-ones row in the
   activation tile (ones written by memset, bias row packed into the weights).
 - transposed convs are convs with flipped kernels over zero-padded tiles
   (padding pre-zeroed by memsets, tanh writes the interior).
 - the nearest-neighbor upsample 5->13 is done by 4 tanh activations with
   broadcast (step-0) access patterns reading PSUM directly.
 - psi = atan2(qz,qw) - atan2(-qx,qy) uses the half-angle identity
   atan2(y,x) = 2*atan(y/(sqrt(x^2+y^2)+x)); the two atan terms feed emb1 as
   a separate 2-row accumulating matmul with 2x-scaled weights.
All compute-engine SBUF access patterns start at partition 0 (hardware
constraint); DMA handles the few mid-partition placements.
"""

import sys

for _p in ("/opt/trn_rl_repo",):
    if _p not in sys.path:
        sys.path.insert(0, _p)

import numpy as np

import concourse.bacc as bacc
import concourse.bass as bass
import concourse.mybir as mybir
import concourse.tile as tile
from concourse.bass_utils import run_bass_kernel_spmd

F32 = mybir.dt.float32
AF = mybir.ActivationFunctionType
N_CORES = 8


def _ds_mat():
    # PyTorch AdaptiveAvgPool1d(13 -> 5)
    M = np.zeros((5, 13), np.float32)
    for i in range(5):
        s = (i * 13) // 5
        e = -((-(i + 1) * 13) // 5)
        M[i, s:e] = 1.0 / (e - s)
    return M


# const tile column layout (37 partitions x 89 cols)
_C_W1R = 0    # [37, 6]  conv1 weights (row 3i+k = w1[o,i,k]), row36 = bias
_C_DST = 6    # [13, 5]  downsample matrix transposed
_C_W2 = 11    # [7, 12]  conv2, 3 k-blocks of 4; row6 k=0 block = bias
_C_W3 = 23    # [5, 12]  conv3, 3 k-blocks of 4; row4 k=0 block = bias
_C_WE1A = 35  # [6, 4]   emb1 (fm_links 4 rows, x100 row, bias row)
_C_WE1B = 39  # [2, 4]   emb1 psi part: both rows 2*w[:,4]
_C_WE2 = 43   # [5, 4]   emb2 + bias row
_C_WD1 = 47   # [5, 12]  dec1, 3 j-blocks of 4; row4 every block = bias
_C_WD2 = 59   # [5, 6]   dec2 flipped, 3 k-blocks of 2; row4 k=0 = bias
_C_WD3 = 65   # [3, 6]   dec3 flipped; row2 k=0 = bias
_C_WD4 = 71   # [15, 18] dec4 flipped; row14 k=0 = bias
_C_COLS = 89


def _pack(inputs):
    g = lambda k: np.asarray(inputs[k], np.float32)
    x = g("x").reshape(189)

    cat = np.zeros(180, np.float32)
    cat[2:90] = x[7:95]
    cat[92:180] = x[101:189]

    qw, qx, qy, qz = x[3], x[4], x[5], x[6]
    aux = np.array([[qz, qw], [qx, qy], [x[100], 0.0]], np.float32)

    C = np.zeros((37, _C_COLS), np.float32)
    C[0:36, _C_W1R:_C_W1R + 6] = g("conv1_w").transpose(1, 2, 0).reshape(36, 6)
    C[36, _C_W1R:_C_W1R + 6] = g("conv1_b")
    C[0:13, _C_DST:_C_DST + 5] = _ds_mat().T
    C[0:6, _C_W2:_C_W2 + 12] = g("conv2_w").transpose(1, 2, 0).reshape(6, 12)
    C[6, _C_W2:_C_W2 + 4] = g("conv2_b")
    C[0:4, _C_W3:_C_W3 + 12] = g("conv3_w").transpose(1, 2, 0).reshape(4, 12)
    C[4, _C_W3:_C_W3 + 4] = g("conv3_b")
    we1 = g("emb1_w")[:, :, 0]  # (4 out, 6 in)
    C[0:4, _C_WE1A:_C_WE1A + 4] = we1[:, 0:4].T
    C[4, _C_WE1A:_C_WE1A + 4] = we1[:, 5]
    C[5, _C_WE1A:_C_WE1A + 4] = g("emb1_b")
    C[0:2, _C_WE1B:_C_WE1B + 4] = np.broadcast_to(we1[:, 4], (2, 4))
    C[0:4, _C_WE2:_C_WE2 + 4] = g("emb2_w")[:, :, 0].T
    C[4, _C_WE2:_C_WE2 + 4] = g("emb2_b")
    C[0:4, _C_WD1:_C_WD1 + 12] = g("dec1_w").transpose(0, 2, 1).reshape(4, 12)
    C[4, _C_WD1:_C_WD1 + 12] = np.tile(g("dec1_b"), 3)
    C[0:4, _C_WD2:_C_WD2 + 6] = g("dec2_w")[:, :, ::-1].transpose(0, 2, 1).reshape(4, 6)
    C[4, _C_WD2:_C_WD2 + 2] = g("dec2_b")
    C[0:2, _C_WD3:_C_WD3 + 6] = g("dec3_w")[:, :, ::-1].transpose(0, 2, 1).reshape(2, 6)
    C[2, _C_WD3:_C_WD3 + 2] = g("dec3_b")
    C[0:14, _C_WD4:_C_WD4 + 18] = g("dec4_w")[:, :, ::-1].transpose(0, 2, 1).reshape(14, 18)
    C[14, _C_WD4:_C_WD4 + 6] = g("dec4_b")
    return {"cat": cat, "aux": aux, "wconst": C}


def _fap(ap2d, dims):
    """Replace the free dims of a 2D AP with an explicit [step,count] list."""
    n = ap2d.copy()
    n.ap = mybir.VecI64Pair([list(ap2d.ap[0])] + [list(d) for d in dims])
    return n


def _build():
    nc = bacc.Bacc("TRN2", target_bir_lowering=False, debug=False)
    d_cat = nc.dram_tensor("cat", (180,), F32, kind="ExternalInput")
    d_aux = nc.dram_tensor("aux", (3, 2), F32, kind="ExternalInput")
    d_const = nc.dram_tensor("wconst", (37, _C_COLS), F32, kind="ExternalInput")
    d_out = nc.dram_tensor("out", (90,), F32, kind="ExternalOutput")

    with tile.TileContext(nc) as tc:
        with (
            tc.tile_pool(name="sb", bufs=1) as sb,
            tc.tile_pool(name="ps", bufs=4, space=bass.MemorySpace.PSUM) as ps,
        ):
            C = sb.tile([37, _C_COLS], F32, tag="C")
            patch = sb.tile([37, 13], F32, tag="patch")
            fm1T = sb.tile([13, 6], F32, tag="fm1T")
            ds_s = sb.tile([7, 5], F32, tag="ds_s")
            fm2 = sb.tile([5, 3], F32, tag="fm2")
            embA = sb.tile([6, 1], F32, tag="embA")
            scr = sb.tile([2, 17], F32, tag="scr")
            e1s = sb.tile([5, 1], F32, tag="e1s")
            e2s = sb.tile([5, 1], F32, tag="e2s")
            pd1 = sb.tile([5, 7], F32, tag="pd1")
            pus = sb.tile([3, 17], F32, tag="pus")
            d4in = sb.tile([15, 17], F32, tag="d4in")
            outs = sb.tile([6, 15], F32, tag="outs")

            # --- init: ones rows first (whole tile <- 1), then zero interiors
            nc.gpsimd.memset(patch[:, :], 1.0)   # row36 stays 1 (conv1 bias)
            nc.gpsimd.memset(d4in[:, :], 1.0)    # row14 stays 1
            nc.gpsimd.memset(d4in[0:14, :], 0.0)
            nc.gpsimd.memset(ds_s[:, :], 1.0)    # row6
            nc.gpsimd.memset(fm2[:, :], 1.0)     # row4
            nc.gpsimd.memset(embA[:, :], 1.0)    # row5
            nc.gpsimd.memset(e1s[:, :], 1.0)     # row4
            nc.gpsimd.memset(e2s[:, :], 1.0)     # row4
            nc.gpsimd.memset(pd1[:, :], 1.0)     # row4
            nc.gpsimd.memset(pd1[0:4, :], 0.0)
            nc.gpsimd.memset(pus[:, :], 1.0)     # row2
            nc.gpsimd.memset(pus[0:2, :], 0.0)

            # --- input DMAs
            nc.sync.dma_start(C[:, :], d_const.ap())
            catf = d_cat.ap().flatten()
            src1 = catf.copy()  # im2col: patch[3i+k, l] = cat[15i + k + l]
            src1.ap = mybir.VecI64Pair([[15, 12], [1, 3], [1, 13]])
            nc.sync.dma_start(patch[0:36, :], src1)
            src2 = catf.copy()  # jcat rows for dec4 input
            src2.ap = mybir.VecI64Pair([[15, 12], [1, 15]])
            nc.sync.dma_start(d4in[2:14, 1:16], src2)
            nc.sync.dma_start(scr[0:2, 0:2], d_aux.ap()[0:2, :])
            nc.sync.dma_start(embA[4:5, 0:1], d_aux.ap()[2:3, 0:1])

            # --- psi side-chain: theta_i = atan2(y, x) per partition, octant-
            # reduced so the ScalarE Arctan input stays in [0, 1].
            # cols: 0=y 1=x 2=|y| 3=|x| 4=mn 5=mx 6=1/mx 7=q 8=r0 9=swap
            #       10=u 11=r1 12=neg 13=v 14=r2 15=sign(y) 16=theta
            Alu = mybir.AluOpType
            nc.scalar.activation(scr[:, 2:4], scr[:, 0:2], AF.Abs)
            nc.vector.tensor_tensor(scr[:, 4:5], scr[:, 2:3], scr[:, 3:4], op=Alu.min)
            nc.vector.tensor_max(scr[:, 5:6], scr[:, 2:3], scr[:, 3:4])
            nc.vector.reciprocal(scr[:, 6:7], scr[:, 5:6])
            nc.vector.tensor_mul(scr[:, 7:8], scr[:, 4:5], scr[:, 6:7])
            nc.scalar.activation(scr[:, 8:9], scr[:, 7:8], AF.Arctan)
            nc.vector.tensor_tensor(scr[:, 9:10], scr[:, 2:3], scr[:, 3:4], op=Alu.is_gt)
            nc.vector.scalar_tensor_tensor(
                scr[:, 10:11], scr[:, 9:10], float(np.pi / 2), scr[:, 8:9],
                op0=Alu.mult, op1=Alu.subtract)
            nc.scalar.activation(scr[:, 11:12], scr[:, 10:11], AF.Abs)
            nc.vector.tensor_scalar(scr[:, 12:13], scr[:, 1:2], 0.0, None, op0=Alu.is_lt)
            nc.vector.scalar_tensor_tensor(
                scr[:, 13:14], scr[:, 12:13], float(np.pi), scr[:, 11:12],
                op0=Alu.mult, op1=Alu.subtract)
            nc.scalar.activation(scr[:, 14:15], scr[:, 13:14], AF.Abs)
            nc.scalar.activation(scr[:, 15:16], scr[:, 0:1], AF.Sign)
            nc.vector.tensor_mul(scr[:, 16:17], scr[:, 14:15], scr[:, 15:16])

            # --- encoder
            p1 = ps.tile([13, 6], F32, tag="acc")  # conv1^T out
            nc.tensor.matmul(p1[:, :], patch[:, :], C[0:37, _C_W1R:_C_W1R + 6],
                             start=True, stop=True)
            nc.scalar.activation(fm1T[:, :], p1[:, :], AF.Tanh)

            p2 = ps.tile([6, 5], F32, tag="acc")  # downsample out [ch, 5]
            nc.tensor.matmul(p2[:, :], fm1T[:, :], C[0:13, _C_DST:_C_DST + 5],
                             start=True, stop=True)
            nc.vector.tensor_copy(ds_s[0:6, :], p2[:, :])

            p3 = ps.tile([4, 3], F32, tag="acc")  # conv2
            for k in range(3):
                nc.tensor.matmul(p3[:, :], C[0:7, _C_W2 + 4 * k:_C_W2 + 4 * k + 4],
                                 ds_s[0:7, k:k + 3], start=(k == 0), stop=(k == 2))
            nc.scalar.activation(fm2[0:4, :], p3[:, :], AF.Tanh)

            p4 = ps.tile([4, 1], F32, tag="acc")  # conv3 (fm_links)
            for k in range(3):
                nc.tensor.matmul(p4[:, :], C[0:5, _C_W3 + 4 * k:_C_W3 + 4 * k + 4],
                                 fm2[0:5, k:k + 1], start=(k == 0), stop=(k == 2))
            nc.scalar.activation(embA[0:4, 0:1], p4[:, :], AF.Tanh)

            # --- embedding head (psi folded in via second matmul)
            p5 = ps.tile([4, 1], F32, tag="acc")
            nc.tensor.matmul(p5[:, :], C[0:6, _C_WE1A:_C_WE1A + 4],
                             embA[0:6, 0:1], start=True, stop=False)
            nc.tensor.matmul(p5[:, :], C[0:2, _C_WE1B:_C_WE1B + 4],
                             scr[0:2, 16:17], start=False, stop=True)
            nc.scalar.activation(e1s[0:4, 0:1], p5[:, :], AF.Tanh)

            p6 = ps.tile([4, 1], F32, tag="acc")
            nc.tensor.matmul(p6[:, :], C[0:5, _C_WE2:_C_WE2 + 4],
                             e1s[0:5, 0:1], start=True, stop=True)
            nc.scalar.activation(e2s[0:4, 0:1], p6[:, :], AF.Tanh)

            # --- decoder
            p7 = ps.tile([4, 3], F32, tag="acc")  # dec1: one matmul per column
            for j in range(3):
                nc.tensor.matmul(p7[:, j:j + 1], C[0:5, _C_WD1 + 4 * j:_C_WD1 + 4 * j + 4],
                                 e2s[0:5, 0:1], start=True, stop=True)
            nc.scalar.activation(pd1[0:4, 2:5], p7[:, :], AF.Tanh)

            p8 = ps.tile([2, 5], F32, tag="acc")  # dec2
            for k in range(3):
                nc.tensor.matmul(p8[:, :], C[0:5, _C_WD2 + 2 * k:_C_WD2 + 2 * k + 2],
                                 pd1[0:5, k:k + 5], start=(k == 0), stop=(k == 2))
            # nearest upsample 5->13 fused into tanh via broadcast APs
            # UP = [0,0,0,1,1,1,2,2,3,3,3,4,4]; pus col t+2 <- fm_dc2 col UP[t]
            nc.scalar.activation(_fap(pus[0:2, 2:3], [[3, 2], [1, 3]]),
                                 _fap(p8[0:2, 0:1], [[1, 2], [0, 3]]), AF.Tanh)
            nc.scalar.activation(_fap(pus[0:2, 8:9], [[1, 2]]),
                                 _fap(p8[0:2, 2:3], [[0, 2]]), AF.Tanh)
            nc.scalar.activation(_fap(pus[0:2, 10:11], [[1, 3]]),
                                 _fap(p8[0:2, 3:4], [[0, 3]]), AF.Tanh)
            nc.scalar.activation(_fap(pus[0:2, 13:14], [[1, 2]]),
                                 _fap(p8[0:2, 4:5], [[0, 2]]), AF.Tanh)

            p9 = ps.tile([2, 15], F32, tag="acc")  # dec3
            for k in range(3):
                nc.tensor.matmul(p9[:, :], C[0:3, _C_WD3 + 2 * k:_C_WD3 + 2 * k + 2],
                                 pus[0:3, k:k + 15], start=(k == 0), stop=(k == 2))
            nc.scalar.activation(d4in[0:2, 1:16], p9[:, :], AF.Tanh)

            p10 = ps.tile([6, 15], F32, tag="acc")  # dec4 (no activation)
            for k in range(3):
                nc.tensor.matmul(p10[:, :], C[0:15, _C_WD4 + 6 * k:_C_WD4 + 6 * k + 6],
                                 d4in[0:15, k:k + 15], start=(k == 0), stop=(k == 2))

            nc.scalar.activation(outs[:, :], p10[:, :], AF.Copy)
            dst = d_out.ap().flatten().copy()
            dst.ap = mybir.VecI64Pair([[15, 6], [1, 15]])
            nc.sync.dma_start(dst, outs[:, :])

    nc.compile()
    return nc


_NC = None


def _get_nc():
    global _NC
    if _NC is None:
        _NC = _build()
    return _NC


def _run(inputs, trace=False, **kw):
    nc = _get_nc()
    in_map = _pack(inputs)
    res = run_bass_kernel_spmd(
        nc, [in_map] * N_CORES, core_ids=list(range(N_CORES)), trace=trace, **kw
    )
    out90 = np.asarray(res.results[0]["out"], np.float32)
    return out90[2:].reshape(1, 88), res


def kernel(**inputs) -> np.ndarray:
    out, _ = _run(inputs, trace=False)
    return out


def _simulate(inputs):
    """Local CoreSim check (no hardware)."""
    from concourse.bass_interp import CoreSim

    nc = _build()
    sim = CoreSim(nc, trace=False)
    for k, v in _pack(inputs).items():
        sim.tensor(k)[:] = v
    sim.simulate(check_with_hw=False)
    return sim.tensor("out")[2:].reshape(1, 88).copy()
